# revision 29
# baseline (speedup 1.0000x reference)
"""Causal multi-head attention (B=4, S=2048, D=1024, H=16) on 8 Trainium2 cores.

Sharding: core c handles batch b = c//2 and head-half hh = c%2 (8 heads, 512
head-dims). QKV/out projections are tensor-parallel over the head dim;
attention is embarrassingly parallel over (b, head). The out-projection
partial products (rank-512 each) are summed pairwise on the host along with
the output bias.

On-device layout is fully transposed (d_model / head-dim on partitions,
sequence on the free axis) so every matmul contracts over the partition dim
with no on-chip transposes:
  Q^T = (Wq*scale)-tiles.T @ x^T      K^T likewise     V natural = x^T.T @ Wv
  S^T[k,q] = K_h @ Q_h^T              exp on ScalarE (no max subtraction:
                                      |scores| <~ 5, exp is safe in fp32)
  [l..l; O^T] = [1s | V_h].T @ expS^T (32 ones columns in the stationary make
                                      PV emit the softmax denominator
                                      replicated on partitions 0..31)
  O_norm^T = O^T * (1/l)              Y^T-partial = Wo-tiles.T @ O_cat^T
Causality: fully-masked key-blocks are skipped; diagonal blocks compute only
the valid column range and apply one 128x128 triangular mask multiply.
Head pairs (even/odd) interleave so K=64 score matmuls pack into disjoint
PE row-groups and run concurrently.

Schedule notes (v2):
  - ScalarE exp demand inside an attention block exceeds the PE demand, so
    the PE needs independent projection work to chew on while exp drains.
    Scores, PV accumulators and projection accumulators live in separate
    PSUM pools (4+2+2 banks) so projection matmuls are never blocked on a
    score buffer held by ScalarE.
  - ~14 warm-up matmuls on a zeroed tile run at t=0 so the PE HAM clock
    gate lifts (1.2 -> 2.4 GHz) during the DMA lead-in instead of ~35us in;
    a few more zero-dep matmuls at the tail of the program act as idle
    filler wherever the scheduler finds a PE bubble.
  - Input DMA order: biases/mask first, then V/Q/K weights interleaved
    per k-block on one queue while x^T streams (split in column halves)
    on the other; Wo last.  Output y^T chunks drain via ScalarE/VectorE
    copies (alternating) and leave on two DMA queues.
"""

import numpy as np
import ml_dtypes

B, S, D = 4, 2048, 1024
H = 16
HH = 8          # heads per core
DK = 64
HD = 512        # head dims per core
N_CORES = 8
SCALE = DK ** -0.5
PB = 128        # partition block
QB = 512        # query block (matmul free dim)
NQB = S // QB   # 4
NKB = S // PB   # 16
KD = D // PB    # 8
KO = HD // PB   # 4
N_WARM = 32     # leading warm-up matmuls: HAM un-throttle + cover
                # the HBM-BW-bound input DMA lead-in (~27us) so the PE never
                # idles (and re-throttles) before real matmuls are fed
SM_BQ = 0       # column offsets in the merged small-constants tensor
SM_BK = KO
SM_BVB = 2 * KO
SM_MK = 2 * KO + HD
SM_ID = 2 * KO + HD + 2 * PB
SM_W = 2 * KO + HD + 3 * PB   # 904 bf16 cols = 1808 B/partition

_COMPILED = None
LAST_RESULTS = None


def _build():
    from contextlib import ExitStack
    import concourse.bass as bass
    import concourse.tile as tile
    from concourse import bacc, mybir

    BF16 = mybir.dt.bfloat16
    F32 = mybir.dt.float32
    AF = mybir.ActivationFunctionType

    nc = bacc.Bacc("TRN2", target_bir_lowering=False, debug=False,
                   num_devices=N_CORES)

    # Host-shuffled layouts: partition-major so each DMA is one fully
    # contiguous transfer (no strided DRAM gathers, ~1us per dma_start).
    xT_d = nc.dram_tensor("xT", [PB, NQB * KD * QB], BF16,
                          kind="ExternalInput")
    wq_d = nc.dram_tensor("wq", [PB, KD * HD], BF16, kind="ExternalInput")
    wk_d = nc.dram_tensor("wk", [PB, KD * HD], BF16, kind="ExternalInput")
    wv_d = nc.dram_tensor("wv", [PB, KD * HD], BF16, kind="ExternalInput")
    wo_d = nc.dram_tensor("wo", [PB, KO * D], BF16, kind="ExternalInput")
    # bq/bk/bvb/mk merged into one tensor: tiny per-partition rows (16 B)
    # DMA at ~50 MB/s (one ~0.5us packet per row per engine); merged rows
    # are 1552 B and move at full rate.
    sm_d = nc.dram_tensor("smalls", [PB, SM_W], BF16, kind="ExternalInput")
    yT_d = nc.dram_tensor("yT", [D, S], F32, kind="ExternalOutput")

    with tile.TileContext(nc) as tc, ExitStack() as ctx:
        persist = ctx.enter_context(tc.tile_pool(name="persist", bufs=1))
        work = ctx.enter_context(tc.tile_pool(name="work", bufs=6))
        nrm = ctx.enter_context(tc.tile_pool(name="nrm", bufs=4))
        # PSUM: 8 banks total.  scores 2x2 + PV accumulators 2x1 +
        # projection accumulators 2x1 = 8.
        psS = ctx.enter_context(tc.tile_pool(name="psS", bufs=2, space="PSUM"))
        psO = ctx.enter_context(tc.tile_pool(name="psO", bufs=2, space="PSUM"))
        psP = ctx.enter_context(tc.tile_pool(name="psP", bufs=2, space="PSUM"))

        # xT layout [p, qb, k, s']: one contiguous 1MB DMA per query block,
        # and matmul operands slice out contiguous [PB, 512] / [PB, 128] runs.
        xT_t = persist.tile([PB, NQB, KD, QB], BF16, name="xT")
        wq_t = persist.tile([PB, KD, HD], BF16, name="wq")
        wk_t = persist.tile([PB, KD, HD], BF16, name="wk")
        wv_t = persist.tile([PB, KD, HD], BF16, name="wv")
        wo_t = persist.tile([PB, KO, D], BF16, name="wo")
        wq = [wq_t[:, k, :] for k in range(KD)]
        wk = [wk_t[:, k, :] for k in range(KD)]
        wv = [wv_t[:, k, :] for k in range(KD)]
        wo = [wo_t[:, k, :] for k in range(KO)]

        def x_qb(k, qb):
            # [PB, QB] slice of x^T for d_model block k, query block qb
            return xT_t[:, qb, k, :]

        def x_kb(k, kb):
            # [PB, PB] slice of x^T for d_model block k, seq block kb
            return xT_t[:, kb // 4, k, (kb % 4) * PB:(kb % 4 + 1) * PB]
        sm = persist.tile([PB, SM_W], BF16, name="smalls")
        bvb = sm[:, SM_BVB:SM_BVB + HD]
        mk = sm[:, SM_MK:SM_MK + 2 * PB].rearrange("p (j c) -> p j c", j=2)
        ident = sm[:, SM_ID:SM_ID + PB]
        # tensor_scalar bias operands must be f32: one-time widen from sm
        bq = persist.tile([PB, KO], F32, name="bq")
        bk = persist.tile([PB, KO], F32, name="bk")
        qT = [persist.tile([PB, S], BF16, name=f"qT{k}") for k in range(KO)]
        kT = [persist.tile([PB, S], BF16, name=f"kT{k}") for k in range(KO)]
        # per key-block: 8 heads x (32 ones columns + 64 V columns).
        # Ones first so the PV matmul puts the softmax denominator on
        # partitions 0..31 (reciprocal_approx_fast requires base partition 0).
        # 96 stationary columns instead of 128 shaves ~25% off every PV
        # LDWEIGHTS (weight-load time scales with column count).
        ON = 32
        VW = ON + DK     # 96
        v = [persist.tile([PB, HH, VW], BF16, name=f"v{k}") for k in range(NKB)]
        onorm = [persist.tile([PB, S], BF16, name=f"onorm{k}") for k in range(KO)]
        wsrc = persist.tile([PB, QB], BF16, name="wsrc")

        # --- warm-up: PE busy from t=0 so HAM un-throttles early ---------
        nc.vector.memset(wsrc[:], 0.0)

        def warm_mm(cols=QB):
            wps = psP.tile([PB, QB], F32, tag="p")
            nc.tensor.matmul(wps[:, 0:cols], wsrc[:, 0:PB], wsrc[:, 0:cols],
                             start=True, stop=True)

        for _ in range(N_WARM):
            warm_mm()

        # --- input DMA on three queues (sync/scalar/gpsimd are the only
        # DMA-capable engines).  HBM read BW is the startup gate (~330 GB/s
        # aggregate, first byte ~2us after descriptor push at ~7us): the
        # kernel's first real matmuls need {x-qb0, wq, wk} = 3 MB, so those
        # lead the three queues (x0/wq additionally split in halves so the
        # k<4 accumulation steps can start ~4us earlier).  Everything else
        # queues behind them.
        for kb in range(NKB):
            nc.vector.memset(v[kb][:, :, 0:ON], 1.0)
        QW = KD * QB

        def x_dma(eng, q):
            eng.dma_start(
                xT_t[:, q, :, :],
                xT_d[:, q * QW:(q + 1) * QW].rearrange("p (k s) -> p k s",
                                                       k=KD))

        KD2 = KD // 2
        nc.sync.dma_start(
            xT_t[:, 0, 0:KD2, :],
            xT_d[:, 0:QW // 2].rearrange("p (k s) -> p k s", k=KD2))
        nc.scalar.dma_start(wk_t[:],
                            wk_d[:].rearrange("p (k h) -> p k h", k=KD))
        nc.gpsimd.dma_start(
            wq_t[:, 0:KD2, :],
            wq_d[:, 0:KD2 * HD].rearrange("p (k h) -> p k h", k=KD2))
        nc.sync.dma_start(
            xT_t[:, 0, KD2:KD, :],
            xT_d[:, QW // 2:QW].rearrange("p (k s) -> p k s", k=KD2))
        nc.gpsimd.dma_start(
            wq_t[:, KD2:KD, :],
            wq_d[:, KD2 * HD:KD * HD].rearrange("p (k h) -> p k h", k=KD2))
        nc.gpsimd.dma_start(sm[:], sm_d[:])
        nc.vector.tensor_copy(bq[:], sm[:, SM_BQ:SM_BQ + KO])
        nc.vector.tensor_copy(bk[:], sm[:, SM_BK:SM_BK + KO])
        nc.gpsimd.dma_start(wv_t[:],
                            wv_d[:].rearrange("p (k h) -> p k h", k=KD))
        x_dma(nc.sync, 1)
        x_dma(nc.scalar, 2)
        x_dma(nc.sync, 3)
        nc.scalar.dma_start(wo_t[:],
                            wo_d[:].rearrange("p (j d) -> p j d", j=KO))

        # --- V projection, one key-block (128 seq positions) per chunk ---
        def v_gen(kb):
            accv = psP.tile([PB, QB], F32, tag="p")
            for k in range(KD):
                nc.tensor.matmul(accv[:], x_kb(k, kb),
                                 wv[k][:], start=(k == 0), stop=(k == KD - 1),
                                 skip_group_check=True)
                yield
            nc.vector.tensor_add(
                v[kb][:, :, ON:VW],
                accv[:].rearrange("p (h d) -> p h d", h=HH),
                bvb.rearrange("p (h d) -> p h d", h=HH))

        def q_gen(m, qb, w, bias, dst):
            # one q- or k-projection chunk as a generator: one matmul per
            # next(), bias-add drain emitted with the last step
            qs = slice(qb * QB, (qb + 1) * QB)
            acc = psP.tile([PB, QB], F32, tag="p")
            for k in range(KD):
                nc.tensor.matmul(acc[:], w[k][:, m * PB:(m + 1) * PB],
                                 x_qb(k, qb), start=(k == 0),
                                 stop=(k == KD - 1), skip_group_check=True)
                yield
            nc.vector.tensor_scalar_add(dst[m][:, qs], acc[:],
                                        bias[:, m:m + 1])

        def o_gen(mo, qb):
            # one out-projection row-block: 4 matmuls into a 1-bank psP
            # accumulator, VectorE drain (keep ScalarE pure-exp), DMA out
            qs = slice(qb * QB, (qb + 1) * QB)
            y_ps = psP.tile([PB, QB], F32, tag="p")
            for k2 in range(KO):
                nc.tensor.matmul(y_ps[:], wo[k2][:, mo * PB:(mo + 1) * PB],
                                 onorm[k2][:, qs], start=(k2 == 0),
                                 stop=(k2 == KO - 1), skip_group_check=True)
                yield
            y_sb = nrm.tile([PB, QB], F32, tag="y")
            nc.vector.tensor_copy(y_sb[:], y_ps[:])
            eng = nc.sync if (mo + qb) % 2 == 0 else nc.gpsimd
            eng.dma_start(yT_d[mo * PB:(mo + 1) * PB, qs], y_sb[:])

        # qb=3's out-projection gates the kernel tail: split its contraction
        # so the k2<2 half (ready after head-pairs 0/1 normalize) runs inside
        # the last two attention blocks, staged to SBUF (bf16 partial, ~4e-3
        # abs rounding on y -- well inside budget).  The k2>=2 half re-adds
        # the staged partial with an identity matmul and drains on ScalarE
        # (idle after the last exp), keeping VectorE off the tail chain.
        ystage = [persist.tile([PB, QB], BF16, name=f"yst{mo}")
                  for mo in range(KD)]

        def o_gen_a(mo, qb):
            qs = slice(qb * QB, (qb + 1) * QB)
            y_ps = psP.tile([PB, QB], F32, tag="p")
            for k2 in (0, 1):
                nc.tensor.matmul(y_ps[:], wo[k2][:, mo * PB:(mo + 1) * PB],
                                 onorm[k2][:, qs], start=(k2 == 0),
                                 stop=(k2 == 1), skip_group_check=True)
                yield
            nc.vector.tensor_copy(ystage[mo][:], y_ps[:])

        def o_gen_b(mo, qb):
            qs = slice(qb * QB, (qb + 1) * QB)
            y_ps = psP.tile([PB, QB], F32, tag="p")
            for k2 in (2, 3):
                nc.tensor.matmul(y_ps[:], wo[k2][:, mo * PB:(mo + 1) * PB],
                                 onorm[k2][:, qs], start=(k2 == 2),
                                 stop=False, skip_group_check=True)
                yield
            nc.tensor.matmul(y_ps[:], ident, ystage[mo][:], start=False,
                             stop=True, skip_group_check=True)
            yield
            y_sb = nrm.tile([PB, QB], F32, tag="y")
            nc.scalar.copy(y_sb[:], y_ps[:])
            eng = nc.sync if (mo + qb) % 2 == 0 else nc.gpsimd
            eng.dma_start(yT_d[mo * PB:(mo + 1) * PB, qs], y_sb[:])

        # filler queue: (deadline, ready, generator) of single projection
        # matmuls, deadline-ordered; pumped from inside attention blocks so
        # the PE always has a weight-load-friendly matmul while ScalarE
        # drains exp.  `ready` is the ORDER position whose completion the
        # generator's inputs require: pumping past it would emit matmuls
        # that head-of-line-stall the PE queue.
        fq = []
        cur_blk = [0]

        def pump(n=1):
            for _ in range(n):
                while fq:
                    if fq[0][1] > cur_blk[0]:
                        return
                    try:
                        next(fq[0][2])
                        break
                    except StopIteration:
                        fq.pop(0)
                else:
                    break

        def flush_until(pos):
            while fq and fq[0][0] <= pos:
                _, _, g = fq.pop(0)
                for _ in g:
                    pass

        def attention(hp, qb):
            # head pair 2*hp (rows 0:64) + 2*hp+1 (rows 64:128), query block qb.
            # PV runs one kb behind scores so each PV pair's exp has a full
            # score-slot of ScalarE lead time before the PE needs it.
            m = hp
            qs = slice(qb * QB, (qb + 1) * QB)
            nkb = 4 * qb + 4
            o_accs = [psO.tile([PB, QB], F32, tag="oacc", name=f"oacc{i}")
                      for i in range(2)]
            pend = []

            def pv(kb, c0, e_sb):
                for i in range(2):
                    h = 2 * hp + i
                    nc.tensor.matmul(o_accs[i][0:VW, c0:QB],
                                     v[kb][:, h, :], e_sb[:, i, c0:QB],
                                     start=(kb == 0), stop=(kb == nkb - 1),
                                     skip_group_check=True)

            for kb in range(nkb):
                t = kb - 4 * qb
                c0 = 0 if t < 0 else PB * t
                cs = slice(qb * QB + c0, (qb + 1) * QB)
                s_ps = psS.tile([PB, 2, QB], F32, tag="s")
                for i, rb in enumerate((0, DK)):
                    nc.tensor.matmul(
                        s_ps[:, i, c0:QB],
                        kT[m][rb:rb + DK, kb * PB:(kb + 1) * PB],
                        qT[m][rb:rb + DK, cs], start=True, stop=True)
                e_sb = work.tile([PB, 2, QB], BF16, tag="exp")
                nc.scalar.activation(e_sb[:, :, c0:QB], s_ps[:, :, c0:QB],
                                     AF.Exp)
                if t >= 0:
                    nc.vector.tensor_mul(e_sb[:, :, c0:c0 + PB],
                                         e_sb[:, :, c0:c0 + PB], mk)
                pend.append((kb, c0, e_sb))
                pump(1 if qb < 2 else 2)
                if len(pend) > 2:
                    pv(*pend.pop(0))
                pump(1 if qb < 2 else 2)
            while pend:
                pv(*pend.pop(0))
            pump(2)
            # denominator on partitions 0:32, O on 32:96; normalize in two
            # 32-row multiplies against the 32-row reciprocal.
            for i, rb in enumerate((0, DK)):
                r_sb = nrm.tile([ON, QB], F32, tag="r")
                nc.vector.reciprocal_approx_fast(r_sb[:], o_accs[i][0:ON, :])
                nc.vector.tensor_mul(onorm[m][rb:rb + ON, qs],
                                     o_accs[i][ON:2 * ON, :], r_sb[:])
                nc.vector.tensor_mul(onorm[m][rb + ON:rb + DK, qs],
                                     o_accs[i][2 * ON:VW, :], r_sb[:])

        # Block order interleaves cheap (low-qb) and expensive (qb=3)
        # attention blocks so ScalarE exp demand never outruns the PE's
        # attention+filler supply; the first four blocks need only the
        # first DMA wave (wq/wk/x-qb0, then wv).
        ORDER = [(0, 0), (1, 0), (2, 0), (3, 0),
                 (0, 1), (1, 1), (2, 1), (3, 1),
                 (0, 2), (1, 2), (2, 2), (3, 2),
                 (0, 3), (1, 3), (2, 3), (3, 3)]

        def dl_qk(m, q):
            return min(i for i, (hp, qb) in enumerate(ORDER)
                       if hp == m and qb >= q)

        def dl_v(kb):
            return min(i for i, (hp, qb) in enumerate(ORDER)
                       if qb >= kb // 4)

        entries = []
        for m in range(KO):
            for q in range(NQB):
                entries.append((dl_qk(m, q), 2 * (4 * m + q), 0,
                                q_gen(m, q, wq, bq, qT)))
                entries.append((dl_qk(m, q), 2 * (4 * m + q) + 1, 0,
                                q_gen(m, q, wk, bk, kT)))
        for kb in range(NKB):
            entries.append((dl_v(kb), 100 + kb, 0, v_gen(kb)))
        # out-projection chunks: ready once the last head-pair of their
        # query block normalizes; pure filler + tail (deadline infinity).
        # qb=3 is k2-split: phase a ready after head-pair 1 of qb=3; phase b
        # (needing the final block's normalize) is emitted after the loop.
        ready = {q: max(i for i, (hp, qb) in enumerate(ORDER) if qb == q)
                 for q in range(NQB)}
        ready_a3 = max(i for i, (hp, qb) in enumerate(ORDER)
                       if qb == 3 and hp <= 1)
        for q in sorted(range(NQB - 1), key=lambda q: ready[q]):
            for mo in range(KD):
                entries.append((1 << 20, 1000 + ready[q] * 10 + mo,
                                ready[q] + 1, o_gen(mo, q)))
        for mo in range(KD):
            entries.append((1 << 20, 1000 + ready_a3 * 10 + mo,
                            ready_a3 + 1, o_gen_a(mo, 3)))
        entries.sort(key=lambda e: (e[0], e[1]))
        fq.extend((dl, rd, g) for dl, _, rd, g in entries)

        for i, blk in enumerate(ORDER):
            cur_blk[0] = i
            flush_until(i)
            attention(*blk)
        cur_blk[0] = 1 << 30
        flush_until(1 << 30)
        for mo in range(KD):
            for _ in o_gen_b(mo, 3):
                pass

    nc.compile()
    return nc


def _get_compiled():
    global _COMPILED
    if _COMPILED is None:
        _COMPILED = _build()
    return _COMPILED


def _shuf_w(w):
    # [D, HD] -> [PB, KD*HD] with [p, k*HD+h] = w[k*PB+p, h]
    kd = w.shape[0] // PB
    return np.ascontiguousarray(
        w.reshape(kd, PB, w.shape[1]).transpose(1, 0, 2).reshape(PB, -1))


def _shuf_x(xb):
    # [S, D] -> [PB, NQB*KD*QB] with [p, ((q*KD)+k)*QB+s'] = x[q*QB+s', k*PB+p]
    t = xb.reshape(NQB, QB, KD, PB).transpose(3, 0, 2, 1)
    return np.ascontiguousarray(t.reshape(PB, -1))


def _make_in_maps(x, Wq, bq, Wk, bk, Wv, bv, Wo):
    bf16 = ml_dtypes.bfloat16

    # inclusive lower-triangular mask for diagonal 128x128 blocks
    p_idx = np.arange(PB)[:, None]
    c_idx = np.arange(PB)[None, :]
    mk = np.tile((p_idx <= c_idx).astype(bf16), (1, 2))

    in_maps = []
    for c in range(N_CORES):
        b, hh = c // 2, c % 2
        cs = slice(hh * HD, (hh + 1) * HD)
        sm = np.empty((PB, SM_W), dtype=bf16)
        sm[:, SM_BQ:SM_BQ + KO] = (bq[cs] * SCALE).reshape(KO, PB).T
        sm[:, SM_BK:SM_BK + KO] = bk[cs].reshape(KO, PB).T
        sm[:, SM_BVB:SM_BVB + HD] = np.broadcast_to(bv[cs], (PB, HD))
        sm[:, SM_MK:SM_MK + 2 * PB] = mk
        sm[:, SM_ID:SM_ID + PB] = np.eye(PB, dtype=bf16)
        in_maps.append({
            "xT": _shuf_x(x[b].astype(bf16)),
            "wq": _shuf_w((Wq[:, cs] * SCALE).astype(bf16)),
            "wk": _shuf_w(Wk[:, cs].astype(bf16)),
            "wv": _shuf_w(Wv[:, cs].astype(bf16)),
            "wo": _shuf_w(Wo[cs, :].astype(bf16)),
            "smalls": sm,
        })
    return in_maps


def _reference_fallback(x, mask, Wq, bq, Wk, bk, Wv, bv, Wo, bo):
    out = np.empty((B, S, D), dtype=np.float32)
    for b in range(B):
        q = (x[b] @ Wq + bq).reshape(S, H, DK).transpose(1, 0, 2)
        k = (x[b] @ Wk + bk).reshape(S, H, DK).transpose(1, 0, 2)
        vv = (x[b] @ Wv + bv).reshape(S, H, DK).transpose(1, 0, 2)
        o = np.empty((H, S, DK), dtype=np.float32)
        for hi in range(H):
            s = (q[hi] @ k[hi].T) * SCALE
            s = np.where(mask[b], -1e9, s)
            s = s - s.max(axis=-1, keepdims=True)
            e = np.exp(s)
            p = e / e.sum(axis=-1, keepdims=True)
            o[hi] = p @ vv[hi]
        out[b] = o.transpose(1, 0, 2).reshape(S, D) @ Wo + bo
    return out


def kernel(x, mask, Wq, bq, Wk, bk, Wv, bv, Wo, bo, **kwargs):
    global LAST_RESULTS
    import os

    x = np.asarray(x, dtype=np.float32)
    mask = np.asarray(mask)

    causal = np.triu(np.ones((S, S), dtype=bool), k=1)
    if not all(np.array_equal(mask[b], causal) for b in range(B)):
        return _reference_fallback(np.asarray(x), mask, np.asarray(Wq),
                                   np.asarray(bq), np.asarray(Wk),
                                   np.asarray(bk), np.asarray(Wv),
                                   np.asarray(bv), np.asarray(Wo),
                                   np.asarray(bo))

    from concourse.bass_utils import run_bass_kernel_spmd

    nc = _get_compiled()
    in_maps = _make_in_maps(x, np.asarray(Wq), np.asarray(bq), np.asarray(Wk),
                            np.asarray(bk), np.asarray(Wv), np.asarray(bv),
                            np.asarray(Wo))
    trace = bool(int(os.environ.get("KERNEL_PROFILE", "0")))
    res = run_bass_kernel_spmd(nc, in_maps, list(range(N_CORES)), trace=trace)
    LAST_RESULTS = res

    bo32 = np.asarray(bo, dtype=np.float32)
    out = np.empty((B, S, D), dtype=np.float32)
    for b in range(B):
        acc = res.results[2 * b]["yT"] + res.results[2 * b + 1]["yT"]
        out[b] = acc.T + bo32
    return out



# revision 46
# speedup vs baseline: 1.0963x; 1.0963x over previous
"""Causal multi-head attention (B=4, S=2048, D=1024, H=16) on 8 Trainium2 cores.

Sharding: core c handles batch b = c//2 and head-half hh = c%2 (8 heads, 512
head-dims). QKV/out projections are tensor-parallel over the head dim;
attention is embarrassingly parallel over (b, head). The out-projection
partial products (rank-512 each) are summed pairwise on the host along with
the output bias.

On-device layout is fully transposed (d_model / head-dim on partitions,
sequence on the free axis) so every matmul contracts over the partition dim
with no on-chip transposes:
  Q^T = (Wq*scale)-tiles.T @ x^T      K^T likewise     V natural = x^T.T @ Wv
  S^T[k,q] = K_h @ Q_h^T              exp on ScalarE (no max subtraction:
                                      |scores| <~ 5, exp is safe in fp32)
  [l..l; O^T] = [1s | V_h].T @ expS^T (32 ones columns in the stationary make
                                      PV emit the softmax denominator
                                      replicated on partitions 0..31)
  O_norm^T = O^T * (1/l)              Y^T-partial = Wo-tiles.T @ O_cat^T
Causality: fully-masked key-blocks are skipped; diagonal blocks compute only
the valid column range and apply one 128x128 triangular mask multiply.
Head pairs (even/odd) interleave so K=64 score matmuls pack into disjoint
PE row-groups and run concurrently.

Schedule notes (v3):
  - ScalarE exp demand inside an attention block exceeds the PE demand, so
    the PE needs independent projection work to chew on while exp drains.
    Scores, PV accumulators and projection accumulators live in separate
    PSUM pools (4+2+2 banks) so projection matmuls are never blocked on a
    score buffer held by ScalarE.
  - ~32 warm-up matmuls run from ~7us (wsrc memset on the otherwise-idle
    GpSimd queue) so the PE HAM clock gate lifts during the DMA lead-in.
  - Startup is HBM-read-BW bound (~330 GB/s): bq/bk/bvb/mask/identity are
    merged into one fat-row tensor (tiny-row DMAs are packet-latency bound,
    ~8us for 4 KB), and the critical input set {x-qb0, wq, wk, wv, smalls}
    is split/balanced ~1.4 MB per DMA queue so the first four attention
    blocks are fed by ~22us; x-qb1..3 and Wo queue behind it.
  - Projection generators flush two blocks before their consumer: their
    VectorE bias-add drains must clear the DVE FIFO before the consuming
    block's first score LDWEIGHTS, or they queue behind the previous
    block's normalize muls (~3us/block otherwise).
  - The filler pump is readiness-gated (never emits a matmul whose inputs
    need a block that hasn't run -- head-of-line stalls) and pumps double
    in the first two kb-steps of each block, where the first PV must wait
    for ScalarE to drain the previous block's exp backlog.
  - qb=3's out-projection gates the tail, so its contraction is k2-split:
    the k2<2 half runs inside the last two attention blocks and stages to
    SBUF (bf16); after the final block the k2>=2 half re-adds the staged
    partial with an identity matmul and drains via ScalarE copy, keeping
    the busy VectorE off the tail chain.
"""

import numpy as np
import ml_dtypes

B, S, D = 4, 2048, 1024
H = 16
HH = 8          # heads per core
DK = 64
HD = 512        # head dims per core
N_CORES = 8
SCALE = DK ** -0.5
PB = 128        # partition block
QB = 512        # query block (matmul free dim)
NQB = S // QB   # 4
NKB = S // PB   # 16
KD = D // PB    # 8
KO = HD // PB   # 4
N_WARM = 32     # leading warm-up matmuls: HAM un-throttle + cover
                # the HBM-BW-bound input DMA lead-in (~27us) so the PE never
                # idles (and re-throttles) before real matmuls are fed
SM_BQ = 0       # column offsets in the merged small-constants tensor
SM_BK = KO
SM_BVB = 2 * KO
SM_MK = 2 * KO + HD
SM_ID = 2 * KO + HD + 2 * PB
SM_W = 2 * KO + HD + 3 * PB   # 904 bf16 cols = 1808 B/partition

_COMPILED = None
LAST_RESULTS = None


def _build():
    from contextlib import ExitStack
    import concourse.bass as bass
    import concourse.tile as tile
    from concourse import bacc, mybir

    BF16 = mybir.dt.bfloat16
    F32 = mybir.dt.float32
    AF = mybir.ActivationFunctionType

    nc = bacc.Bacc("TRN2", target_bir_lowering=False, debug=False,
                   num_devices=N_CORES)

    # Host-shuffled layouts: partition-major so each DMA is one fully
    # contiguous transfer (no strided DRAM gathers, ~1us per dma_start).
    xT_d = nc.dram_tensor("xT", [PB, NQB * KD * QB], BF16,
                          kind="ExternalInput")
    wq_d = nc.dram_tensor("wq", [PB, KD * HD], BF16, kind="ExternalInput")
    wk_d = nc.dram_tensor("wk", [PB, KD * HD], BF16, kind="ExternalInput")
    wv_d = nc.dram_tensor("wv", [PB, KD * HD], BF16, kind="ExternalInput")
    wo_d = nc.dram_tensor("wo", [PB, KO * D], BF16, kind="ExternalInput")
    # bq/bk/bvb/mk merged into one tensor: tiny per-partition rows (16 B)
    # DMA at ~50 MB/s (one ~0.5us packet per row per engine); merged rows
    # are 1552 B and move at full rate.
    sm_d = nc.dram_tensor("smalls", [PB, SM_W], BF16, kind="ExternalInput")
    yT_d = nc.dram_tensor("yT", [D, S], F32, kind="ExternalOutput")

    with tile.TileContext(nc) as tc, ExitStack() as ctx:
        persist = ctx.enter_context(tc.tile_pool(name="persist", bufs=1))
        work = ctx.enter_context(tc.tile_pool(name="work", bufs=6))
        nrm = ctx.enter_context(tc.tile_pool(name="nrm", bufs=4))
        # PSUM: 8 banks total.  scores 2x2 + PV accumulators 2x1 +
        # projection accumulators 2x1 = 8.
        psS = ctx.enter_context(tc.tile_pool(name="psS", bufs=2, space="PSUM"))
        psO = ctx.enter_context(tc.tile_pool(name="psO", bufs=2, space="PSUM"))
        psP = ctx.enter_context(tc.tile_pool(name="psP", bufs=2, space="PSUM"))

        # xT layout [p, qb, k, s']: one contiguous 1MB DMA per query block,
        # and matmul operands slice out contiguous [PB, 512] / [PB, 128] runs.
        xT_t = persist.tile([PB, NQB, KD, QB], BF16, name="xT")
        wq_t = persist.tile([PB, KD, HD], BF16, name="wq")
        wk_t = persist.tile([PB, KD, HD], BF16, name="wk")
        wv_t = persist.tile([PB, KD, HD], BF16, name="wv")
        wo_t = persist.tile([PB, KO, D], BF16, name="wo")
        wq = [wq_t[:, k, :] for k in range(KD)]
        wk = [wk_t[:, k, :] for k in range(KD)]
        wv = [wv_t[:, k, :] for k in range(KD)]
        wo = [wo_t[:, k, :] for k in range(KO)]

        def x_qb(k, qb):
            # [PB, QB] slice of x^T for d_model block k, query block qb
            return xT_t[:, qb, k, :]

        def x_kb(k, kb):
            # [PB, PB] slice of x^T for d_model block k, seq block kb
            return xT_t[:, kb // 4, k, (kb % 4) * PB:(kb % 4 + 1) * PB]
        sm = persist.tile([PB, SM_W], BF16, name="smalls")
        bvb = sm[:, SM_BVB:SM_BVB + HD]
        mk = sm[:, SM_MK:SM_MK + 2 * PB].rearrange("p (j c) -> p j c", j=2)
        ident = sm[:, SM_ID:SM_ID + PB]
        # tensor_scalar bias operands must be f32: one-time widen from sm
        bq = persist.tile([PB, KO], F32, name="bq")
        bk = persist.tile([PB, KO], F32, name="bk")
        qT = [persist.tile([PB, S], BF16, name=f"qT{k}") for k in range(KO)]
        kT = [persist.tile([PB, S], BF16, name=f"kT{k}") for k in range(KO)]
        # per key-block: 8 heads x (32 ones columns + 64 V columns).
        # Ones first so the PV matmul puts the softmax denominator on
        # partitions 0..31 (reciprocal_approx_fast requires base partition 0).
        # 96 stationary columns instead of 128 shaves ~25% off every PV
        # LDWEIGHTS (weight-load time scales with column count).
        ON = 32
        VW = ON + DK     # 96
        v = [persist.tile([PB, HH, VW], BF16, name=f"v{k}") for k in range(NKB)]
        onorm = [persist.tile([PB, S], BF16, name=f"onorm{k}") for k in range(KO)]
        wsrc = persist.tile([PB, QB], BF16, name="wsrc")

        # --- warm-up: PE busy from t=0 so HAM un-throttles early.  memset
        # on GpSimd: its queue is idle at program start, so the first warm
        # matmul's weight load isn't stuck behind the VectorE preamble.
        nc.gpsimd.memset(wsrc[:], 0.0)

        def warm_mm(cols=QB):
            wps = psP.tile([PB, QB], F32, tag="p")
            nc.tensor.matmul(wps[:, 0:cols], wsrc[:, 0:PB], wsrc[:, 0:cols],
                             start=True, stop=True)

        for _ in range(N_WARM):
            warm_mm()

        # --- input DMA on three queues (sync/scalar/gpsimd are the only
        # DMA-capable engines).  HBM read BW is the startup gate (~330 GB/s
        # aggregate, first byte ~2us after descriptor push at ~7us): the
        # kernel's first real matmuls need {x-qb0, wq, wk} = 3 MB, so those
        # lead the three queues (x0/wq additionally split in halves so the
        # k<4 accumulation steps can start ~4us earlier).  Everything else
        # queues behind them.
        for kb in range(NKB):
            nc.vector.memset(v[kb][:, :, 0:ON], 1.0)
        QW = KD * QB

        def x_dma(eng, q):
            eng.dma_start(
                xT_t[:, q, :, :],
                xT_d[:, q * QW:(q + 1) * QW].rearrange("p (k s) -> p k s",
                                                       k=KD))

        KD2 = KD // 2
        nc.sync.dma_start(
            xT_t[:, 0, 0:KD2, :],
            xT_d[:, 0:QW // 2].rearrange("p (k s) -> p k s", k=KD2))
        nc.scalar.dma_start(wk_t[:],
                            wk_d[:].rearrange("p (k h) -> p k h", k=KD))
        nc.gpsimd.dma_start(
            wq_t[:, 0:KD2, :],
            wq_d[:, 0:KD2 * HD].rearrange("p (k h) -> p k h", k=KD2))
        nc.sync.dma_start(
            xT_t[:, 0, KD2:KD, :],
            xT_d[:, QW // 2:QW].rearrange("p (k s) -> p k s", k=KD2))
        nc.gpsimd.dma_start(
            wq_t[:, KD2:KD, :],
            wq_d[:, KD2 * HD:KD * HD].rearrange("p (k h) -> p k h", k=KD2))
        nc.gpsimd.dma_start(sm[:], sm_d[:])
        nc.vector.tensor_copy(bq[:], sm[:, SM_BQ:SM_BQ + KO])
        nc.vector.tensor_copy(bk[:], sm[:, SM_BK:SM_BK + KO])
        # wv split across the two other queues: the critical set
        # {x0, wq, wk, wv, sm} = 4.2 MB balances ~1.4 MB/queue (the queues
        # split HBM BW about evenly), landing everything the first four
        # attention blocks need by ~22us; x1-x3/wo follow.
        nc.sync.dma_start(
            wv_t[:, 0:KD2, :],
            wv_d[:, 0:KD2 * HD].rearrange("p (k h) -> p k h", k=KD2))
        nc.scalar.dma_start(
            wv_t[:, KD2:KD, :],
            wv_d[:, KD2 * HD:KD * HD].rearrange("p (k h) -> p k h", k=KD2))
        x_dma(nc.sync, 1)
        x_dma(nc.scalar, 2)
        x_dma(nc.gpsimd, 3)
        nc.scalar.dma_start(wo_t[:],
                            wo_d[:].rearrange("p (j d) -> p j d", j=KO))

        # --- V projection, one key-block (128 seq positions) per chunk ---
        def v_gen(kb):
            accv = psP.tile([PB, QB], F32, tag="p")
            for k in range(KD):
                nc.tensor.matmul(accv[:], x_kb(k, kb),
                                 wv[k][:], start=(k == 0), stop=(k == KD - 1),
                                 skip_group_check=True)
                yield
            nc.vector.tensor_add(
                v[kb][:, :, ON:VW],
                accv[:].rearrange("p (h d) -> p h d", h=HH),
                bvb.rearrange("p (h d) -> p h d", h=HH))

        def q_gen(m, qb, w, bias, dst):
            # one q- or k-projection chunk as a generator: one matmul per
            # next(), bias-add drain emitted with the last step
            qs = slice(qb * QB, (qb + 1) * QB)
            acc = psP.tile([PB, QB], F32, tag="p")
            for k in range(KD):
                nc.tensor.matmul(acc[:], w[k][:, m * PB:(m + 1) * PB],
                                 x_qb(k, qb), start=(k == 0),
                                 stop=(k == KD - 1), skip_group_check=True)
                yield
            nc.vector.tensor_scalar_add(dst[m][:, qs], acc[:],
                                        bias[:, m:m + 1])

        def o_gen(mo, qb):
            # one out-projection row-block: 4 matmuls into a 1-bank psP
            # accumulator, VectorE drain (keep ScalarE pure-exp), DMA out
            qs = slice(qb * QB, (qb + 1) * QB)
            y_ps = psP.tile([PB, QB], F32, tag="p")
            for k2 in range(KO):
                nc.tensor.matmul(y_ps[:], wo[k2][:, mo * PB:(mo + 1) * PB],
                                 onorm[k2][:, qs], start=(k2 == 0),
                                 stop=(k2 == KO - 1), skip_group_check=True)
                yield
            y_sb = nrm.tile([PB, QB], F32, tag="y")
            nc.vector.tensor_copy(y_sb[:], y_ps[:])
            eng = nc.sync if (mo + qb) % 2 == 0 else nc.gpsimd
            eng.dma_start(yT_d[mo * PB:(mo + 1) * PB, qs], y_sb[:])

        # qb=3's out-projection gates the kernel tail: split its contraction
        # so the k2<2 half (ready after head-pairs 0/1 normalize) runs inside
        # the last two attention blocks, staged to SBUF (bf16 partial, ~4e-3
        # abs rounding on y -- well inside budget).  The k2>=2 half re-adds
        # the staged partial with an identity matmul and drains on ScalarE
        # (idle after the last exp), keeping VectorE off the tail chain.
        ystage = [persist.tile([PB, QB], BF16, name=f"yst{mo}")
                  for mo in range(KD)]

        def o_gen_a(mo, qb):
            qs = slice(qb * QB, (qb + 1) * QB)
            y_ps = psP.tile([PB, QB], F32, tag="p")
            for k2 in (0, 1):
                nc.tensor.matmul(y_ps[:], wo[k2][:, mo * PB:(mo + 1) * PB],
                                 onorm[k2][:, qs], start=(k2 == 0),
                                 stop=(k2 == 1), skip_group_check=True)
                yield
            nc.vector.tensor_copy(ystage[mo][:], y_ps[:])

        def o_gen_b(mo, qb):
            qs = slice(qb * QB, (qb + 1) * QB)
            y_ps = psP.tile([PB, QB], F32, tag="p")
            for k2 in (2, 3):
                nc.tensor.matmul(y_ps[:], wo[k2][:, mo * PB:(mo + 1) * PB],
                                 onorm[k2][:, qs], start=(k2 == 2),
                                 stop=False, skip_group_check=True)
                yield
            nc.tensor.matmul(y_ps[:], ident, ystage[mo][:], start=False,
                             stop=True, skip_group_check=True)
            yield
            y_sb = nrm.tile([PB, QB], F32, tag="y")
            nc.scalar.copy(y_sb[:], y_ps[:])
            eng = nc.sync if (mo + qb) % 2 == 0 else nc.gpsimd
            eng.dma_start(yT_d[mo * PB:(mo + 1) * PB, qs], y_sb[:])

        # filler queue: (deadline, ready, generator) of single projection
        # matmuls, deadline-ordered; pumped from inside attention blocks so
        # the PE always has a weight-load-friendly matmul while ScalarE
        # drains exp.  `ready` is the ORDER position whose completion the
        # generator's inputs require: pumping past it would emit matmuls
        # that head-of-line-stall the PE queue.
        fq = []
        cur_blk = [0]

        def pump(n=1):
            for _ in range(n):
                idx = 0
                while idx < len(fq):
                    if fq[idx][1] > cur_blk[0]:
                        idx += 1
                        continue
                    try:
                        next(fq[idx][2])
                        break
                    except StopIteration:
                        fq.pop(idx)
                else:
                    break

        def flush_until(pos):
            while fq and fq[0][0] <= pos:
                _, _, g = fq.pop(0)
                for _ in g:
                    pass

        def attention(hp, qb):
            # head pair 2*hp (rows 0:64) + 2*hp+1 (rows 64:128), query block qb.
            # PV runs one kb behind scores so each PV pair's exp has a full
            # score-slot of ScalarE lead time before the PE needs it.
            m = hp
            qs = slice(qb * QB, (qb + 1) * QB)
            nkb = 4 * qb + 4
            o_accs = [psO.tile([PB, QB], F32, tag="oacc", name=f"oacc{i}")
                      for i in range(2)]
            pend = []

            def pv(kb, c0, e_sb):
                for i in range(2):
                    h = 2 * hp + i
                    nc.tensor.matmul(o_accs[i][0:VW, c0:QB],
                                     v[kb][:, h, :], e_sb[:, i, c0:QB],
                                     start=(kb == 0), stop=(kb == nkb - 1),
                                     skip_group_check=True)

            for kb in range(nkb):
                t = kb - 4 * qb
                c0 = 0 if t < 0 else PB * t
                cs = slice(qb * QB + c0, (qb + 1) * QB)
                s_ps = psS.tile([PB, 2, QB], F32, tag="s")
                for i, rb in enumerate((0, DK)):
                    nc.tensor.matmul(
                        s_ps[:, i, c0:QB],
                        kT[m][rb:rb + DK, kb * PB:(kb + 1) * PB],
                        qT[m][rb:rb + DK, cs], start=True, stop=True)
                e_sb = work.tile([PB, 2, QB], BF16, tag="exp")
                nc.scalar.activation(e_sb[:, :, c0:QB], s_ps[:, :, c0:QB],
                                     AF.Exp)
                if t >= 0:
                    nc.vector.tensor_mul(e_sb[:, :, c0:c0 + PB],
                                         e_sb[:, :, c0:c0 + PB], mk)
                pend.append((kb, c0, e_sb))
                # extra filler early in the block: the first PV must wait for
                # ScalarE to drain the previous block's exp backlog (~2 calls)
                pump(2 if kb < 2 else 1)
                if len(pend) > 1:
                    pv(*pend.pop(0))
                pump(1)
            while pend:
                pv(*pend.pop(0))
            pump(2)
            # denominator on partitions 0:32, O on 32:96; normalize in two
            # 32-row multiplies against the 32-row reciprocal.
            for i, rb in enumerate((0, DK)):
                r_sb = nrm.tile([ON, QB], F32, tag="r")
                nc.vector.reciprocal_approx_fast(r_sb[:], o_accs[i][0:ON, :])
                nc.vector.tensor_mul(onorm[m][rb:rb + ON, qs],
                                     o_accs[i][ON:2 * ON, :], r_sb[:])
                nc.vector.tensor_mul(onorm[m][rb + ON:rb + DK, qs],
                                     o_accs[i][2 * ON:VW, :], r_sb[:])

        # Block order interleaves cheap (low-qb) and expensive (qb=3)
        # attention blocks so ScalarE exp demand never outruns the PE's
        # attention+filler supply; the first four blocks need only the
        # first DMA wave (wq/wk/x-qb0, then wv).
        ORDER = [(0, 0), (1, 0), (2, 0), (3, 0),
                 (0, 1), (1, 1), (2, 1), (3, 1),
                 (0, 2), (1, 2), (2, 2), (3, 2),
                 (0, 3), (1, 3), (2, 3), (3, 3)]

        def dl_qk(m, q):
            # one block of slack: the drain (bias add) must clear the DVE
            # queue before the consuming block's first score LDWEIGHTS, and
            # the previous block's normalize muls otherwise sit ahead of it
            return max(0, min(i for i, (hp, qb) in enumerate(ORDER)
                              if hp == m and qb >= q) - 2)

        def dl_v(kb):
            return max(0, min(i for i, (hp, qb) in enumerate(ORDER)
                              if qb >= kb // 4) - 2)

        entries = []
        for m in range(KO):
            for q in range(NQB):
                entries.append((dl_qk(m, q), 2 * (4 * m + q), 0,
                                q_gen(m, q, wq, bq, qT)))
                entries.append((dl_qk(m, q), 2 * (4 * m + q) + 1, 0,
                                q_gen(m, q, wk, bk, kT)))
        for kb in range(NKB):
            entries.append((dl_v(kb), 100 + kb, 0, v_gen(kb)))
        # out-projection chunks: ready once the last head-pair of their
        # query block normalizes; pure filler + tail (deadline infinity).
        # qb=3 is k2-split: phase a ready after head-pair 1 of qb=3; phase b
        # (needing the final block's normalize) is emitted after the loop.
        ready = {q: max(i for i, (hp, qb) in enumerate(ORDER) if qb == q)
                 for q in range(NQB)}
        ready_a3 = max(i for i, (hp, qb) in enumerate(ORDER)
                       if qb == 3 and hp <= 1)
        for q in sorted(range(NQB - 1), key=lambda q: ready[q]):
            for mo in range(KD):
                entries.append((1 << 20, 1000 + ready[q] * 10 + mo,
                                ready[q] + 1, o_gen(mo, q)))
        for mo in range(KD):
            entries.append((1 << 20, 1000 + ready_a3 * 10 + mo,
                            ready_a3 + 1, o_gen_a(mo, 3)))
        entries.sort(key=lambda e: (e[0], e[1]))
        fq.extend((dl, rd, g) for dl, _, rd, g in entries)

        for i, blk in enumerate(ORDER):
            cur_blk[0] = i
            flush_until(i)
            attention(*blk)
        cur_blk[0] = 1 << 30
        flush_until(1 << 30)
        for mo in range(KD):
            for _ in o_gen_b(mo, 3):
                pass

    nc.compile()
    return nc


def _get_compiled():
    global _COMPILED
    if _COMPILED is None:
        _COMPILED = _build()
    return _COMPILED


def _shuf_w(w):
    # [D, HD] -> [PB, KD*HD] with [p, k*HD+h] = w[k*PB+p, h]
    kd = w.shape[0] // PB
    return np.ascontiguousarray(
        w.reshape(kd, PB, w.shape[1]).transpose(1, 0, 2).reshape(PB, -1))


def _shuf_x(xb):
    # [S, D] -> [PB, NQB*KD*QB] with [p, ((q*KD)+k)*QB+s'] = x[q*QB+s', k*PB+p]
    t = xb.reshape(NQB, QB, KD, PB).transpose(3, 0, 2, 1)
    return np.ascontiguousarray(t.reshape(PB, -1))


def _make_in_maps(x, Wq, bq, Wk, bk, Wv, bv, Wo):
    bf16 = ml_dtypes.bfloat16

    # inclusive lower-triangular mask for diagonal 128x128 blocks
    p_idx = np.arange(PB)[:, None]
    c_idx = np.arange(PB)[None, :]
    mk = np.tile((p_idx <= c_idx).astype(bf16), (1, 2))

    in_maps = []
    for c in range(N_CORES):
        b, hh = c // 2, c % 2
        cs = slice(hh * HD, (hh + 1) * HD)
        sm = np.empty((PB, SM_W), dtype=bf16)
        sm[:, SM_BQ:SM_BQ + KO] = (bq[cs] * SCALE).reshape(KO, PB).T
        sm[:, SM_BK:SM_BK + KO] = bk[cs].reshape(KO, PB).T
        sm[:, SM_BVB:SM_BVB + HD] = np.broadcast_to(bv[cs], (PB, HD))
        sm[:, SM_MK:SM_MK + 2 * PB] = mk
        sm[:, SM_ID:SM_ID + PB] = np.eye(PB, dtype=bf16)
        in_maps.append({
            "xT": _shuf_x(x[b].astype(bf16)),
            "wq": _shuf_w((Wq[:, cs] * SCALE).astype(bf16)),
            "wk": _shuf_w(Wk[:, cs].astype(bf16)),
            "wv": _shuf_w(Wv[:, cs].astype(bf16)),
            "wo": _shuf_w(Wo[cs, :].astype(bf16)),
            "smalls": sm,
        })
    return in_maps


def _reference_fallback(x, mask, Wq, bq, Wk, bk, Wv, bv, Wo, bo):
    out = np.empty((B, S, D), dtype=np.float32)
    for b in range(B):
        q = (x[b] @ Wq + bq).reshape(S, H, DK).transpose(1, 0, 2)
        k = (x[b] @ Wk + bk).reshape(S, H, DK).transpose(1, 0, 2)
        vv = (x[b] @ Wv + bv).reshape(S, H, DK).transpose(1, 0, 2)
        o = np.empty((H, S, DK), dtype=np.float32)
        for hi in range(H):
            s = (q[hi] @ k[hi].T) * SCALE
            s = np.where(mask[b], -1e9, s)
            s = s - s.max(axis=-1, keepdims=True)
            e = np.exp(s)
            p = e / e.sum(axis=-1, keepdims=True)
            o[hi] = p @ vv[hi]
        out[b] = o.transpose(1, 0, 2).reshape(S, D) @ Wo + bo
    return out


def kernel(x, mask, Wq, bq, Wk, bk, Wv, bv, Wo, bo, **kwargs):
    global LAST_RESULTS
    import os

    x = np.asarray(x, dtype=np.float32)
    mask = np.asarray(mask)

    causal = np.triu(np.ones((S, S), dtype=bool), k=1)
    if not all(np.array_equal(mask[b], causal) for b in range(B)):
        return _reference_fallback(np.asarray(x), mask, np.asarray(Wq),
                                   np.asarray(bq), np.asarray(Wk),
                                   np.asarray(bk), np.asarray(Wv),
                                   np.asarray(bv), np.asarray(Wo),
                                   np.asarray(bo))

    from concourse.bass_utils import run_bass_kernel_spmd

    nc = _get_compiled()
    in_maps = _make_in_maps(x, np.asarray(Wq), np.asarray(bq), np.asarray(Wk),
                            np.asarray(bk), np.asarray(Wv), np.asarray(bv),
                            np.asarray(Wo))
    trace = bool(int(os.environ.get("KERNEL_PROFILE", "0")))
    res = run_bass_kernel_spmd(nc, in_maps, list(range(N_CORES)), trace=trace)
    LAST_RESULTS = res

    bo32 = np.asarray(bo, dtype=np.float32)
    out = np.empty((B, S, D), dtype=np.float32)
    for b in range(B):
        acc = res.results[2 * b]["yT"] + res.results[2 * b + 1]["yT"]
        out[b] = acc.T + bo32
    return out

